# revision 34
# baseline (speedup 1.0000x reference)
# Self-contained Trainium2 Bass kernel for the LN->QKV->sparse-rel-pos-attention->proj block.
#
# Reference computation (B=128, N=256, DIM=512, H=12, KD=32, D=128):
#   xn   = LayerNorm(x) * gamma + beta
#   qkv  = xn @ Wqkv + bqkv ; split q,k,v per head
#   attn = softmax(q k^T / sqrt(KD) + biases[:, bias_idxs])
#   out  = (attn @ v) @ Wproj + bproj
#
# Strategy: pure data-parallel over batch across 8 NeuronCores (16 elems/core).
#
# v2 design vs the f32r baseline (574us -> ~455-530us HW):
#  - qk matmul: single-fp8-e4m3 operands with DoubleRow perf mode packing two
#    K=128 tiles per instruction (HW runs a DR inst at the same rows/cycle as
#    a K=128 fp16 inst, so DR halves the qk instruction count). Per-tensor
#    power-of-2 prescaling keeps e4m3 out of its subnormal range; measured
#    rel err 1.45e-2 vs the 2e-2 gate (error budget spent deliberately here).
#  - v/S/AV/Z/proj matmuls in fp16 (1.0 cycles/row, same rate as f32r but
#    faster weight loads; fp8-residual R3 variants exist behind mode flags
#    but lose to fp16 on instruction count since DR is not 0.5 rows on HW).
#  - Softmax normalizer via ones-[128x128] matmul -> Z broadcast across all
#    partitions in PSUM; reciprocal_approx_fast on DVE (exact reciprocal is
#    ~3.3us/call!). No DRAM partition-broadcast round trip.
#  - x -> z^T transpose on the DMA xbar (dma_start_transpose), not the PE.
#  - LN stats batched (pair 0 + rest) so both Sqrt activations precede the
#    first Exp: 2 act-table loads total (each costs 1.3us).
#  - expb bias multiply split DVE/GpSimd; v drains split Act/DVE; bulk DMAs
#    (weights, x prefetch, y stores) issue from the gpsimd queue so the sync
#    queue only carries the latency-critical z^T transposes.

import numpy as np

B, N, DIM = 128, 256, 512
H, KD = 12, 32
D = 128
DH = D * H
RES = 16
EPS = 1e-5
NCORES = 8
BPC = B // NCORES

_CACHE = {}

# heads processed in strip-pure pairs: strips (h % 3) equal within each pair
HEAD_ORDER = [0, 3, 6, 9, 1, 4, 7, 10, 2, 5, 8, 11]


def _build(bpc, qk_mode, v_mode, proj_mode, eq, ek, ev, ep, use_bqk, use_bp):
    from contextlib import ExitStack

    import concourse.bacc as bacc
    import concourse.tile as tile
    from concourse import mybir

    f32 = mybir.dt.float32
    f16 = mybir.dt.float16
    f8 = mybir.dt.float8e4
    Alu = mybir.AluOpType
    Act = mybir.ActivationFunctionType
    DR = mybir.MatmulPerfMode.DoubleRow

    nc = bacc.Bacc("TRN2", target_bir_lowering=False, debug=False,
                   num_devices=NCORES)

    x_d = nc.dram_tensor("x", [bpc, N, DIM], f32, kind="ExternalInput").ap()
    if qk_mode == "r3":
        wqkh_d = nc.dram_tensor("wqkh", [128, 4, 2, 8 * 128], f8,
                                kind="ExternalInput").ap()
        wqkl_d = nc.dram_tensor("wqkl", [128, 4, 8 * 128], f8,
                                kind="ExternalInput").ap()
    elif qk_mode == "q8":
        wqk8_d = nc.dram_tensor("wqk8", [128, 4, 8 * 128], f8,
                                kind="ExternalInput").ap()
    else:
        wqk_d = nc.dram_tensor("wqk", [128, 4, 8 * 128], f16,
                               kind="ExternalInput").ap()
    if v_mode == "r3":
        wvh_d = nc.dram_tensor("wvh", [128, 4, 2, DH], f8,
                               kind="ExternalInput").ap()
        wvl_d = nc.dram_tensor("wvl", [128, 4, DH], f8,
                               kind="ExternalInput").ap()
    else:
        wv_d = nc.dram_tensor("wv", [128, 4, DH], f16,
                              kind="ExternalInput").ap()
    if proj_mode == "r3":
        wph_d = nc.dram_tensor("wph", [128, 2, H, DIM], f8,
                               kind="ExternalInput").ap()
        wpl_d = nc.dram_tensor("wpl", [128, H, DIM], f8,
                               kind="ExternalInput").ap()
    else:
        wp_d = nc.dram_tensor("wp", [128, H, DIM], f16,
                              kind="ExternalInput").ap()
    expb_d = nc.dram_tensor("expb", [128, 2, H, N], f16,
                            kind="ExternalInput").ap()
    ones_d = nc.dram_tensor("ones", [128, 128], f16, kind="ExternalInput").ap()
    if use_bqk:
        bqk_d = nc.dram_tensor("bqk", [128, 8], f32, kind="ExternalInput").ap()
    if use_bp:
        bp_d = nc.dram_tensor("bp", [DIM], f32, kind="ExternalInput").ap()
    y_d = nc.dram_tensor("y", [bpc, N, DIM], f32, kind="ExternalOutput").ap()

    # drain scales: z carries 2^4; weights carry 2^eq/2^ek/2^ev/2^ep;
    # softmax normalizer ones carry 2^-9 so otn = 512*O.
    sc_q = 2.0 ** -(4 + eq)
    sc_k = 2.0 ** -(4 + ek)
    sc_v = 2.0 ** -(4 + ev)
    sc_y = 2.0 ** -(9 + ep) if proj_mode == "r3" else 2.0 ** -9

    with tile.TileContext(nc) as tc, ExitStack() as ctx:
        consts = ctx.enter_context(tc.tile_pool(name="consts", bufs=1))
        sb_x = ctx.enter_context(tc.tile_pool(name="sb_x", bufs=bpc // 2))
        sb_stat = ctx.enter_context(tc.tile_pool(name="sb_stat", bufs=1))
        sb_st6 = ctx.enter_context(tc.tile_pool(name="sb_st6", bufs=4))
        sb_xn = ctx.enter_context(tc.tile_pool(name="sb_xn", bufs=4))
        sb_zT = ctx.enter_context(tc.tile_pool(name="sb_zT", bufs=2))
        sb_z8 = ctx.enter_context(tc.tile_pool(name="sb_z8", bufs=2))
        sb_qkT = ctx.enter_context(tc.tile_pool(name="sb_qkT", bufs=2))
        sb_v = ctx.enter_context(tc.tile_pool(name="sb_v", bufs=2))
        sb_pt = ctx.enter_context(tc.tile_pool(name="sb_pt", bufs=2))
        sb_zb = ctx.enter_context(tc.tile_pool(name="sb_zb", bufs=2))
        sb_ot = ctx.enter_context(tc.tile_pool(name="sb_ot", bufs=2))
        sb_y = ctx.enter_context(tc.tile_pool(name="sb_y", bufs=2))
        ps_work = ctx.enter_context(tc.tile_pool(name="ps_work", bufs=2, space="PSUM"))
        ps_s = ctx.enter_context(tc.tile_pool(name="ps_s", bufs=2, space="PSUM"))
        # z and ot share one double-buffered pool (alternating per g);
        # the freed bank double-buffers y so proj never stalls on its drain
        ps_ot = ctx.enter_context(tc.tile_pool(name="ps_ot", bufs=2, space="PSUM"))
        ps_z = ps_ot
        ps_y = ctx.enter_context(tc.tile_pool(name="ps_y", bufs=2, space="PSUM"))

        # ---- pair-0 x load first (gpsimd DMA queue issues in ~25ns vs
        # 565ns on sync; sync queue stays clear for z^T transposes) ----
        eps_t = consts.tile([128, 1], f32)
        nc.vector.memset(eps_t, EPS / 256.0)
        xp = [None]
        for p in range(1, bpc // 2):
            x_tile = sb_x.tile([128, 2, 2, DIM], f32, tag="x")
            xp.append(x_tile)
        # pair 0 streams in per (e,tc) chunk so its LN chain starts ~4us in
        xc = []
        for i in range(4):
            x_c = sb_x.tile([128, DIM], f32, tag="xc")
            nc.sync.dma_start(out=x_c,
                              in_=x_d[i // 2, (i % 2) * 128:(i % 2) * 128 + 128, :])
            xc.append(x_c)

        # ---- constants ----
        if qk_mode == "r3":
            wqkh_sb = consts.tile([128, 4, 2, 8 * 128], f8)
            nc.gpsimd.dma_start(out=wqkh_sb, in_=wqkh_d)
            wqkl_sb = consts.tile([128, 4, 8 * 128], f8)
            nc.gpsimd.dma_start(out=wqkl_sb, in_=wqkl_d)
        elif qk_mode == "q8":
            wqk8_sb = consts.tile([128, 4, 8 * 128], f8)
            nc.gpsimd.dma_start(out=wqk8_sb, in_=wqk8_d)
        else:
            wqk_sb = consts.tile([128, 4, 8 * 128], f16)
            nc.gpsimd.dma_start(out=wqk_sb, in_=wqk_d)
        if v_mode == "r3":
            wvh_sb = consts.tile([128, 4, 2, DH], f8)
            nc.gpsimd.dma_start(out=wvh_sb, in_=wvh_d)
            wvl_sb = consts.tile([128, 4, DH], f8)
            nc.gpsimd.dma_start(out=wvl_sb, in_=wvl_d)
        else:
            wv_sb = consts.tile([128, 4, DH], f16)
            nc.gpsimd.dma_start(out=wv_sb, in_=wv_d)
        if proj_mode == "r3":
            wph_sb = consts.tile([128, 2, H, DIM], f8)
            nc.gpsimd.dma_start(out=wph_sb, in_=wph_d)
            wpl_sb = consts.tile([128, H, DIM], f8)
            nc.gpsimd.dma_start(out=wpl_sb, in_=wpl_d)
        else:
            wp_sb = consts.tile([128, H, DIM], f16)
            nc.gpsimd.dma_start(out=wp_sb, in_=wp_d)
        expb_sb = consts.tile([128, 2, H, N], f16)
        nc.gpsimd.dma_start(out=expb_sb, in_=expb_d)
        ones_sb = consts.tile([128, 128], f16)
        nc.gpsimd.dma_start(out=ones_sb, in_=ones_d)
        if use_bqk:
            bqk_sb = consts.tile([128, 8], f32)
            nc.gpsimd.dma_start(out=bqk_sb, in_=bqk_d)
        if use_bp:
            bp_sb = consts.tile([128, 1, DIM], f32)
            nc.gpsimd.dma_start(out=bp_sb, in_=bp_d.partition_broadcast(128))

        # ---- phase A: LN stats split into pair-0 + rest so the main loop
        # starts early while both Sqrt batches still precede the first Exp
        # (2 act-table loads total) ----
        for p in range(1, bpc // 2):
            nc.gpsimd.dma_start(
                out=xp[p],
                in_=x_d[2 * p:2 * p + 2].rearrange("e (tc q) d -> q e tc d",
                                                   q=128))

        def _xin(e, tci):
            if e < 2:
                return xc[2 * e + tci]
            return xp[e // 2][:, e % 2, tci, :]

        def _stats(erange, mv_t, sig_t, rsig_t):
            for i, e in enumerate(erange):
                for tci in range(2):
                    st = sb_st6.tile([128, 6], f32, tag="st")
                    nc.vector.bn_stats(st, _xin(e, tci))
                    nc.vector.bn_aggr(mv_t[:, i, tci, :], st)
            # sig/16 = sqrt((var+eps)/256); rsig = 16/sig
            nc.scalar.activation(sig_t, mv_t[:, :, :, 1], Act.Sqrt,
                                 bias=eps_t, scale=1.0 / 256.0)
            nc.vector.reciprocal_approx_fast(out=rsig_t, in_=sig_t)

        mvc = []
        rsigc = []
        for i in range(4):
            st = sb_st6.tile([128, 6], f32, tag="st")
            nc.vector.bn_stats(st, xc[i])
            mv_c = sb_stat.tile([128, 2], f32)
            nc.vector.bn_aggr(mv_c, st)
            sig_c = sb_stat.tile([128, 1], f32)
            nc.scalar.activation(sig_c, mv_c[:, 1:2], Act.Sqrt,
                                 bias=eps_t, scale=1.0 / 256.0)
            rsig_c = sb_stat.tile([128, 1], f32)
            nc.vector.reciprocal_approx_fast(out=rsig_c, in_=sig_c)
            mvc.append(mv_c)
            rsigc.append(rsig_c)
        mvr = sb_stat.tile([128, bpc - 2, 2, 2], f32)
        sigr = sb_stat.tile([128, bpc - 2, 2], f32)
        rsigr = sb_stat.tile([128, bpc - 2, 2], f32)

        def ln_scalars(e, tci):
            if e < 2:
                return mvc[2 * e + tci][:, 0:1], rsigc[2 * e + tci][:, 0:1]
            return (mvr[:, e - 2, tci, 0:1], rsigr[:, e - 2, tci:tci + 1])

        assert bpc % 2 == 0
        any_r3_zw = (qk_mode == "r3") or (v_mode == "r3")
        assert not (qk_mode == "q8" and any_r3_zw)
        for ep in range(bpc // 2):
            # ---- LN apply -> fp16, DMA-xbar transpose to z^T (pair) ----
            zT16 = sb_zT.tile([128, 4, 2 * N], f16, tag="zT")
            for el in range(2):
                e = 2 * ep + el
                for tci in range(2):
                    xn16 = sb_xn.tile([128, DIM], f16, tag="xn")
                    mu_s, rs_s = ln_scalars(e, tci)
                    nc.vector.tensor_scalar(out=xn16, in0=_xin(e, tci),
                                            scalar1=mu_s, scalar2=rs_s,
                                            op0=Alu.subtract, op1=Alu.mult)
                    off = el * N + tci * 128
                    nc.sync.dma_start_transpose(
                        out=zT16[:, :, off:off + 128], in_=xn16)
            if any_r3_zw:
                z8t = sb_z8.tile([128, 4, 2, 2 * N], f8, tag="z8")
                nc.scalar.activation(z8t[:, :, 0, :], zT16, Act.Copy)
                nc.vector.tensor_tensor(out=z8t[:, :, 1, :], in0=zT16,
                                        in1=z8t[:, :, 0, :], op=Alu.subtract)
            elif qk_mode == "q8":
                z8t = sb_z8.tile([128, 4, 2 * N], f8, tag="z8")
                nc.scalar.activation(z8t, zT16, Act.Copy)

            # ---- qk^T [feat, tok-pair]; head h: q in chunk h//3, k in
            # chunk 4+h//3, strip base (h%3)*32. Per-fc tiles: tile deps are
            # tile-granular, so S only waits on its own two chunk drains ----
            qkTs = []
            for fc in range(8):
                qkT_c = sb_qkT.tile([128, 2 * N], f16, tag=f"qkT{fc}")
                qkTs.append(qkT_c)
            for fc in range(8):
                qk_ps = ps_work.tile([128, 512], f32, tag="work")
                if qk_mode == "r3":
                    for kc in range(4):
                        nc.tensor.matmul(qk_ps,
                                         lhsT=wqkh_sb[:, kc, :,
                                                      fc * 128:(fc + 1) * 128],
                                         rhs=z8t[:, kc, :, :],
                                         start=(kc == 0), stop=False,
                                         perf_mode=DR)
                    for j in range(2):
                        nc.tensor.matmul(qk_ps,
                                         lhsT=wqkl_sb[:, 2 * j:2 * j + 2,
                                                      fc * 128:(fc + 1) * 128],
                                         rhs=z8t[:, 2 * j:2 * j + 2, 0, :],
                                         start=False, stop=(j == 1),
                                         perf_mode=DR)
                elif qk_mode == "q8":
                    for j in range(2):
                        nc.tensor.matmul(qk_ps,
                                         lhsT=wqk8_sb[:, 2 * j:2 * j + 2,
                                                      fc * 128:(fc + 1) * 128],
                                         rhs=z8t[:, 2 * j:2 * j + 2, :],
                                         start=(j == 0), stop=(j == 1),
                                         perf_mode=DR)
                else:
                    for kc in range(4):
                        nc.tensor.matmul(qk_ps,
                                         lhsT=wqk_sb[:, kc,
                                                     fc * 128:(fc + 1) * 128],
                                         rhs=zT16[:, kc, :],
                                         start=(kc == 0), stop=(kc == 3))
                nc.scalar.activation(qkTs[fc], qk_ps, Act.Copy,
                                     scale=(sc_q if fc < 4 else sc_k))
                if use_bqk:
                    nc.vector.tensor_scalar_add(
                        out=qkTs[fc], in0=qkTs[fc],
                        scalar1=bqk_sb[:, fc:fc + 1])

            # ---- v = z Wv for both elements first, so the PE has queued
            # work while the attention phase waits on DVE/Act drains ----
            v16s = []
            for el in range(2):
                etok = el * N
                vtiles = [[None] * 3, [None] * 3]
                v16s.append(vtiles)
                for mc in range(2):
                    off = etok + mc * 128
                    for ns in range(3):
                        v_c = sb_v.tile([128, 512], f16, tag=f"v{mc}{ns}")
                        vtiles[mc][ns] = v_c
                        v_ps = ps_work.tile([128, 512], f32, tag="work")
                        if v_mode == "r3":
                            for kc in range(4):
                                nc.tensor.matmul(
                                    v_ps,
                                    lhsT=z8t[:, kc, :, off:off + 128],
                                    rhs=wvh_sb[:, kc, :,
                                               ns * 512:(ns + 1) * 512],
                                    start=(kc == 0), stop=False, perf_mode=DR)
                            for j in range(2):
                                nc.tensor.matmul(
                                    v_ps,
                                    lhsT=z8t[:, 2 * j:2 * j + 2, 0,
                                             off:off + 128],
                                    rhs=wvl_sb[:, 2 * j:2 * j + 2,
                                               ns * 512:(ns + 1) * 512],
                                    start=False, stop=(j == 1), perf_mode=DR)
                        else:
                            for kc in range(4):
                                nc.tensor.matmul(
                                    v_ps,
                                    lhsT=zT16[:, kc, off:off + 128],
                                    rhs=wv_sb[:, kc, ns * 512:(ns + 1) * 512],
                                    start=(kc == 0), stop=(kc == 3))
                        if ns == 1:
                            nc.vector.tensor_scalar_mul(
                                out=v_c, in0=v_ps, scalar1=sc_v)
                        else:
                            nc.scalar.activation(v_c, v_ps, Act.Copy,
                                                 scale=sc_v)

            if ep == 0:
                # stats for the remaining pairs: DVE fills these in while the
                # PE chews on pair-0 qk/v; the second Sqrt still precedes the
                # first Exp in Act program order.
                _stats(range(2, bpc), mvr, sigr, rsigr)

            # ---- attention: el0/el1 interleaved per head-pair g so each
            # engine always has a second independent stream to hide the
            # serial softmax chain (S -> exp -> mult -> Z -> recip -> ot) ----
            if proj_mode == "r3":
                ot_a = sb_ot.tile([128, 2, H, N], f8, tag="ot8")
                ot_b = sb_ot.tile([128, 2, H, N], f8, tag="ot8")
            else:
                ot_a = sb_ot.tile([128, H, N], f16, tag="ot")
                ot_b = sb_ot.tile([128, H, N], f16, tag="ot")
            ots = [ot_a, ot_b]
            for g in range(6):
                for el in range(2):
                    etok = el * N
                    v16 = v16s[el]
                    pt16 = sb_pt.tile([128, 2, 2, N], f16, tag="pt")
                    for mc in range(2):
                        s_ps = ps_s.tile([128, 512], f32, tag="s")
                        for hl in range(2):
                            h = HEAD_ORDER[2 * g + hl]
                            qc = h // 3
                            base = (h % 3) * KD
                            nc.tensor.matmul(
                                s_ps[:, hl * N:(hl + 1) * N],
                                lhsT=qkTs[4 + qc][base:base + KD,
                                                  etok + mc * 128:etok + (mc + 1) * 128],
                                rhs=qkTs[qc][base:base + KD, etok:etok + N],
                                start=True, stop=True)
                        nc.scalar.activation(pt16[:, mc],
                                             s_ps.rearrange("p (a n) -> p a n",
                                                            a=2),
                                             Act.Exp)
                        eng = nc.gpsimd if (g + mc) % 2 == 0 else nc.vector
                        eng.tensor_tensor(
                            out=pt16[:, mc], in0=pt16[:, mc],
                            in1=expb_sb[:, mc, 2 * g:2 * g + 2, :],
                            op=Alu.mult)
                    # Z broadcast to all partitions: ones[128,128] @ P^T
                    z_ps = ps_z.tile([128, 512], f32, tag="otp")
                    for mc in range(2):
                        nc.tensor.matmul(z_ps, lhsT=ones_sb,
                                         rhs=pt16[:, mc, :, :].rearrange(
                                             "p a n -> p (a n)"),
                                         start=(mc == 0), stop=(mc == 1))
                    zb32 = sb_zb.tile([128, 2, N], f32, tag="zb")
                    nc.vector.reciprocal_approx_fast(
                        out=zb32, in_=z_ps.rearrange("p (a n) -> p a n", a=2))
                    # O^T = v^T P^T, then normalize by zb (= 512/Z)
                    ot_ps = ps_ot.tile([128, 512], f32, tag="otp")
                    for hl in range(2):
                        h = HEAD_ORDER[2 * g + hl]
                        for mc in range(2):
                            nc.tensor.matmul(
                                ot_ps[:, hl * N:(hl + 1) * N],
                                lhsT=v16[mc][h // 4][:, (h % 4) * 128:
                                                     (h % 4) * 128 + 128],
                                rhs=pt16[:, mc, hl, :],
                                start=(mc == 0), stop=(mc == 1))
                    if proj_mode == "r3":
                        otn16 = sb_zb.tile([128, 2, N], f16, tag="otn")
                        nc.vector.tensor_tensor(
                            out=otn16,
                            in0=ot_ps.rearrange("p (a n) -> p a n", a=2),
                            in1=zb32, op=Alu.mult)
                        nc.scalar.activation(ots[el][:, 0, 2 * g:2 * g + 2, :],
                                             otn16, Act.Copy)
                        nc.vector.tensor_tensor(
                            out=ots[el][:, 1, 2 * g:2 * g + 2, :], in0=otn16,
                            in1=ots[el][:, 0, 2 * g:2 * g + 2, :],
                            op=Alu.subtract)
                    else:
                        nc.vector.tensor_tensor(
                            out=ots[el][:, 2 * g:2 * g + 2, :],
                            in0=ot_ps.rearrange("p (a n) -> p a n", a=2),
                            in1=zb32, op=Alu.mult)

            for el in range(2):
                e = 2 * ep + el
                # ---- proj: y = O Wp ----
                for nci in range(2):
                    y_ps = ps_y.tile([128, 512], f32, tag="y")
                    if proj_mode == "r3":
                        for s in range(H):
                            nc.tensor.matmul(
                                y_ps,
                                lhsT=ots[el][:, :, s, nci * 128:(nci + 1) * 128],
                                rhs=wph_sb[:, :, s, :],
                                start=(s == 0), stop=False, perf_mode=DR)
                        for j in range(6):
                            nc.tensor.matmul(
                                y_ps,
                                lhsT=ots[el][:, 0, 2 * j:2 * j + 2,
                                             nci * 128:(nci + 1) * 128],
                                rhs=wpl_sb[:, 2 * j:2 * j + 2, :],
                                start=False, stop=(j == 5), perf_mode=DR)
                    else:
                        for s in range(H):
                            nc.tensor.matmul(
                                y_ps,
                                lhsT=ots[el][:, s, nci * 128:(nci + 1) * 128],
                                rhs=wp_sb[:, s, :],
                                start=(s == 0), stop=(s == H - 1))
                    y32 = sb_y.tile([128, DIM], f32, tag="y32")
                    if use_bp:
                        nc.vector.scalar_tensor_tensor(
                            out=y32, in0=y_ps, scalar=sc_y, in1=bp_sb[:, 0, :],
                            op0=Alu.mult, op1=Alu.add)
                    else:
                        nc.scalar.activation(y32, y_ps, Act.Copy, scale=sc_y)
                    # y store on the gpsimd DMA queue: keeps the sync queue
                    # free so next pair's z^T transposes issue without waiting
                    nc.gpsimd.dma_start(out=y_d[e, nci * 128:(nci + 1) * 128, :],
                                        in_=y32)

    nc.compile()
    return nc


def _pow2exp(a, target=16.0):
    rms = float(np.sqrt(np.mean(np.square(a, dtype=np.float64))))
    return int(np.round(np.log2(target / max(rms, 1e-30))))


def _prepare(inputs, qk_mode, v_mode, proj_mode):
    import ml_dtypes
    e4 = ml_dtypes.float8_e4m3fn

    x = np.ascontiguousarray(np.asarray(inputs["x"], dtype=np.float32))
    gamma = np.asarray(inputs["gamma"], dtype=np.float32)
    beta = np.asarray(inputs["beta"], dtype=np.float32)
    Wqkv = np.asarray(inputs["Wqkv"], dtype=np.float32)
    bqkv = np.asarray(inputs["bqkv"], dtype=np.float32)
    Wproj = np.asarray(inputs["Wproj"], dtype=np.float32)
    bproj = np.asarray(inputs["bproj"], dtype=np.float32)
    biases = np.asarray(inputs["biases"], dtype=np.float32)
    bias_idxs = np.asarray(inputs["bias_idxs"])

    def split8(a):
        hi = np.asarray(a, dtype=e4)
        lo = np.asarray(a - hi.astype(np.float32), dtype=e4)
        return hi, lo

    s = np.float32(KD ** -0.5)
    Wg = Wqkv * gamma[:, None]
    bfull = beta @ Wqkv + bqkv
    Wr = Wg.reshape(DIM, H, 64 + D)
    br = bfull.reshape(H, 64 + D)
    # feature layout: head h -> strip (h%3)*32; q in chunk h//3, k in 4+h//3
    wqk = np.zeros((DIM, 8, 128), dtype=np.float32)
    bqk = np.zeros((8, 128), dtype=np.float32)
    for h in range(H):
        qc, base = h // 3, (h % 3) * KD
        wqk[:, qc, base:base + KD] = Wr[:, h, 0:KD] * s
        wqk[:, 4 + qc, base:base + KD] = Wr[:, h, KD:2 * KD]
        bqk[qc, base:base + KD] = br[h, 0:KD] * s
        bqk[4 + qc, base:base + KD] = br[h, KD:2 * KD]
    eq = _pow2exp(wqk[:, :4][np.abs(wqk[:, :4]) > 0])
    ek = _pow2exp(wqk[:, 4:][np.abs(wqk[:, 4:]) > 0])
    wv = np.ascontiguousarray(Wr[:, :, 2 * KD:].reshape(DIM, DH))
    ev = _pow2exp(wv)
    bv = br[:, 2 * KD:].reshape(DH)
    bp = bproj + bv @ Wproj
    # Wproj in HEAD_ORDER slot order
    wp_slot = Wproj.reshape(H, D, DIM)[HEAD_ORDER].transpose(1, 0, 2)  # [128, H, DIM]
    ep = _pow2exp(wp_slot)
    expb = np.exp(biases[:, bias_idxs])  # [H, N, N]
    expb_t = np.ascontiguousarray(
        expb[HEAD_ORDER].reshape(H, 2, 128, N).transpose(2, 1, 0, 3))

    common = {
        "expb": expb_t.astype(np.float16),
        "ones": np.full((128, 128), 2.0 ** -9, dtype=np.float16),
    }

    # qk weights: scale q/k blocks, chunk-major rows (d = kc*128 + p)
    wqk2 = wqk.reshape(DIM, 8 * 128).copy()
    wqk2[:, :4 * 128] *= 2.0 ** eq
    wqk2[:, 4 * 128:] *= 2.0 ** ek
    wqk_cm = wqk2.reshape(4, 128, 8 * 128).transpose(1, 0, 2)  # [128, 4, 1024]
    if qk_mode == "r3":
        hi, lo = split8(wqk_cm)
        common["wqkh"] = np.ascontiguousarray(
            np.stack([hi, hi], axis=2))  # [128, 4, 2, 1024]
        common["wqkl"] = np.ascontiguousarray(lo)
    elif qk_mode == "q8":
        common["wqk8"] = np.ascontiguousarray(np.asarray(wqk_cm, dtype=e4))
    else:
        common["wqk"] = np.ascontiguousarray(
            (wqk_cm * 2.0 ** -(eq if False else 0)).astype(np.float16))
        # fp16 path keeps scaled weights too (drain rescales)
        common["wqk"] = np.ascontiguousarray(wqk_cm.astype(np.float16))

    wv_cm = (wv * 2.0 ** ev).reshape(4, 128, DH).transpose(1, 0, 2)
    if v_mode == "r3":
        hi, lo = split8(wv_cm)
        common["wvh"] = np.ascontiguousarray(np.stack([hi, hi], axis=2))
        common["wvl"] = np.ascontiguousarray(lo)
    else:
        common["wv"] = np.ascontiguousarray(wv_cm.astype(np.float16))

    if proj_mode == "r3":
        hi, lo = split8(wp_slot * 2.0 ** ep)
        common["wph"] = np.ascontiguousarray(np.stack([hi, hi], axis=1))
        common["wpl"] = np.ascontiguousarray(lo)
    else:
        common["wp"] = np.ascontiguousarray(wp_slot.astype(np.float16))

    use_bqk = bool(np.abs(bqk).max() > 0)
    use_bp = bool(np.abs(bp).max() > 0)
    if use_bqk:
        common["bqk"] = np.ascontiguousarray(bqk.T)  # [128, 8]
    if use_bp:
        common["bp"] = np.ascontiguousarray(bp)

    in_maps = []
    for c in range(NCORES):
        m = dict(common)
        m["x"] = np.ascontiguousarray(x[c * BPC:(c + 1) * BPC])
        in_maps.append(m)
    return in_maps, (eq, ek, ev, ep, use_bqk, use_bp)


def run(inputs, trace=False, qk_mode="q8", v_mode="f16", proj_mode="f16",
        **run_kwargs):
    from concourse.bass_utils import run_bass_kernel_spmd

    in_maps, (eq, ek, ev, ep, use_bqk, use_bp) = _prepare(
        inputs, qk_mode, v_mode, proj_mode)
    key = (BPC, qk_mode, v_mode, proj_mode, eq, ek, ev, ep, use_bqk, use_bp)
    if key not in _CACHE:
        _CACHE[key] = _build(*key)
    nc = _CACHE[key]
    res = run_bass_kernel_spmd(nc, in_maps, core_ids=list(range(NCORES)),
                               trace=trace, **run_kwargs)
    y = np.concatenate([res.results[c]["y"] for c in range(NCORES)], axis=0)
    return y, res


def kernel(**inputs):
    y, _ = run(inputs)
    return y
